# revision 11
# baseline (speedup 1.0000x reference)
"""Multi-head attention (B=8, N=2048, dim=64, heads=8) on 8 Trainium2 cores.

Sharding: batch-parallel — one batch element per NeuronCore, weights
replicated, no collectives. Per-core flash-style attention, fully
SBUF-resident (no HBM intermediates).

Schedule highlights (v3 — single-stream, 3-deep S-tile ring):
- One (head-pair, icx) pass at a time; S tiles rotate through a 3-deep
  PSUM ring (6 banks), so S(k) only waits on exp(k-3): ~2100ns of slack
  vs the ~1900ns S -> exp -> st-free round trip that capped the old
  2-stream layout (2-slot slack = ~1314ns -> 260ns stall per slot).
- za accumulators single-buffered (2 banks, 8 total). At each pass end
  the two za banks are EVICTED to SBUF immediately (one copy on ACT,
  one on DVE, ~540ns each) so the next pass's A@V never waits on the
  full recip+broadcast+normalize chain; that chain runs lazily with
  the normalize multiply on the otherwise-idle GPSIMD engine (SBUF in,
  SBUF out).
- Q^T tiles are pre-scaled by EXP_A*SCALE at the PSUM->SBUF copy
  (free: activation Copy with scale), so the DVE Schraudolph exp is a
  single-op tensor_scalar_add (int16 convert-on-write whose bits ARE
  the bf16 exp); ACT exp uses scale=1/EXP_A. Exp alternates engines
  per slot, ~52% on ACT.
- The softmax denominator rides the A@V matmul as a leading ones
  column of V (PSUM partition 0); z is scaled out of the SBUF-evicted
  copy by gpsimd.
- Tokens processed in a permuted order so the x load runs at 4KB per
  DMA descriptor; the output DMA applies the inverse permutation.
"""
import sys

import numpy as np


def _ensure_path():
    try:
        import concourse  # noqa: F401
    except ImportError:
        for p in (
            "/opt/trn_rl_repo",
            "/root/.axon_site",
            "/root/.axon_site/_ro/trn_rl_repo",
            "/root/.axon_site/_ro/pypackages",
        ):
            if p not in sys.path:
                sys.path.append(p)


_ensure_path()

import concourse.bacc as bacc  # noqa: E402
import concourse.mybir as mybir  # noqa: E402
import concourse.tile as tile  # noqa: E402
from concourse.bass_utils import run_bass_kernel_spmd  # noqa: E402
from concourse.masks import make_identity  # noqa: E402

B, N, D, H = 8, 2048, 64, 8
P = 128
NT = N // P          # 16 n-tiles of 128
IC = N // 512        # 4 query chunks of 512
SCALE = float(D) ** -0.5
F32 = mybir.dt.float32
F32R = mybir.dt.float32r
BF16 = mybir.dt.bfloat16
# Schraudolph bf16 exp on DVE: int16(x*EXP_A + EXP_C) bit-viewed as
# bf16 equals exp(x) to within ~3%; the softmax ratio cancels most of
# it. EXP_C = bf16 exponent bias (127<<7) minus a centering shift that
# zeroes the mean relative error so ACT-exact and DVE-approx key tiles
# mix without bias. The int16 convert-on-write does the rounding.
# Q^T is pre-scaled by EXP_A*SCALE, so S' = EXP_A*SCALE*(q.k): the DVE
# exp is int16(S' + EXP_C) in ONE ALU op; ACT exp is exp(S'/EXP_A).
EXP_A = 128.0 / float(np.log(2.0))
EXP_C = 16256.0 - 6.55
I16 = mybir.dt.int16


def build_program(n_cores=B):
    nc = bacc.Bacc("TRN2", target_bir_lowering=False, debug=False,
                   num_devices=n_cores)
    x_d = nc.dram_tensor("x", [N, D], F32, kind="ExternalInput")
    wqkv_d = nc.dram_tensor("w_qkv", [D, 3 * H * D], F32, kind="ExternalInput")
    wout_d = nc.dram_tensor("w_out", [H * D, D], F32, kind="ExternalInput")
    bout_d = nc.dram_tensor("b_out", [D], F32, kind="ExternalInput")
    out_d = nc.dram_tensor("out", [N, D], F32, kind="ExternalOutput")

    with tile.TileContext(nc) as tc:
        with tc.tile_pool(name="const", bufs=1) as const:
            # identity FIRST on gpsimd (transposes block on it); x loads
            # go on the other queues so ident isn't stuck behind them
            ident = const.tile([P, P], F32, tag="ident")
            make_identity(nc, ident[:])

            xall = const.tile([P, NT, D], F32, tag="xall")
            xr = x_d.ap().rearrange("(p t) d -> p t d", p=P)
            for qi, eng in enumerate((nc.sync, nc.gpsimd, nc.scalar,
                                      nc.sync)):
                eng.dma_start(xall[:, 4 * qi:4 * qi + 4, :],
                              xr[:, 4 * qi:4 * qi + 4, :])

            wsb = const.tile([D, 3 * H * D], F32R, tag="wqkv")
            nc.gpsimd.dma_start(wsb[:], wqkv_d.ap())
            wout_f32 = const.tile([P, 4, D], F32, tag="woutf")
            nc.gpsimd.dma_start(
                wout_f32[:], wout_d.ap().rearrange("(t p) d -> p t d", p=P))
            wout_sb = const.tile([P, 4, D], BF16, tag="wout")
            nc.vector.tensor_copy(wout_sb[:], wout_f32[:])
            b_row = const.tile([1, D], F32, tag="brow")
            nc.sync.dma_start(b_row[:], bout_d.ap().rearrange("(a d) -> a d", a=1))
            b_bc = const.tile([P, D], F32, tag="bbc")
            nc.gpsimd.partition_broadcast(b_bc[:], b_row[:])
            ones3 = const.tile([P, H, 1], F32, tag="ones3")
            nc.gpsimd.memset(ones3[:], 1.0)

            xT = const.tile([D, N], F32R, tag="xT")
            # qk_sb[0..3]: Q^T head-pairs [128, N] PRE-SCALED by
            # EXP_A*SCALE; qk_sb[4..7]: K^T pairs (unscaled)
            qk_sb = [const.tile([P, N], BF16, tag=f"qk{i}", name=f"qk{i}")
                     for i in range(8)]
            # V~ per n-tile: [128, H, 128]; per head: col 0 = ones (so the
            # softmax denominator lands at PSUM partition 0), cols 1-63
            # zero, cols 64-127 = V (z at partitions 64-127). The extra
            # LDWEIGHTS columns hide under the 512-col A@V streams via
            # the PE load-ahead window.
            vt_sb = [const.tile([P, H, P], BF16, tag=f"vt{t}", name=f"vt{t}")
                     for t in range(NT)]
            zT = [const.tile([P, N], BF16, tag=f"zT{i}", name=f"zT{i}")
                  for i in range(4)]

            with (
                tc.tile_pool(name="spsum", bufs=3,
                             space=bacc.bass.MemorySpace.PSUM) as spsum,
                tc.tile_pool(name="zpsum", bufs=1,
                             space=bacc.bass.MemorySpace.PSUM) as zpsum,
                tc.tile_pool(name="es", bufs=8) as es_pool,
                tc.tile_pool(name="sm", bufs=3) as sm_pool,
                tc.tile_pool(name="outp", bufs=3) as outp,
            ):
                # (tokens are processed in the permuted order pi(p,t) =
                # row p*NT+t: each SBUF partition loads one contiguous 4KB
                # block of x. Attention is permutation-equivariant over
                # tokens; the output DMA applies the inverse permutation.)

                def ring_psum():
                    # the 3-deep S-tile ring (6 banks); setup matmuls and
                    # the output projection borrow slots from it
                    return spsum.tile([P, 1024], F32, tag="st",
                                      name="st", bufs=3)

                def emit_k(ct, icxs):
                    w_sl = wsb[:, ct * P:(ct + 1) * P]
                    for icx in icxs:
                        mp = ring_psum()
                        nc.tensor.matmul(
                            mp[0:P, 0:512], w_sl,
                            xT[:, icx * 512:(icx + 1) * 512],
                            start=True, stop=True)
                        nc.scalar.copy(
                            qk_sb[ct][:, icx * 512:(icx + 1) * 512],
                            mp[0:P, 0:512])

                def emit_q(ct, icxs):
                    w_sl = wsb[:, ct * P:(ct + 1) * P]
                    for icx in icxs:
                        mp = ring_psum()
                        nc.tensor.matmul(
                            mp[0:P, 0:512], w_sl,
                            xT[:, icx * 512:(icx + 1) * 512],
                            start=True, stop=True)
                        # pre-scale Q by EXP_A*SCALE (free via activation
                        # Copy scale); DVE exp then needs no multiply
                        nc.scalar.activation(
                            qk_sb[ct][:, icx * 512:(icx + 1) * 512],
                            mp[0:P, 0:512],
                            mybir.ActivationFunctionType.Copy,
                            scale=EXP_A * SCALE)

                def emit_vproj(t):
                    nc.gpsimd.memset(vt_sb[t][:, :, 1:64], 0.0)
                    mp = ring_psum()
                    nc.tensor.matmul(
                        mp[0:P, 0:512], xT[:, t * P:(t + 1) * P],
                        wsb[:, 2 * H * D:3 * H * D],
                        start=True, stop=True)
                    nc.gpsimd.tensor_copy(vt_sb[t][:, :, 0:1], ones3[:])
                    nc.scalar.copy(
                        vt_sb[t][:, :, 64:P],
                        mp[0:P, 0:512].rearrange("p (h d) -> p h d", h=H))

                def emit_transposes(ts):
                    for t in ts:
                        pp = ring_psum()
                        nc.tensor.transpose(pp[0:D, 0:P], xall[:, t, :],
                                            ident[:])
                        nc.vector.tensor_copy(xT[:, t * P:(t + 1) * P],
                                              pp[0:D, 0:P])

                # setup, ordered so pass 0 ((icx0, hp0): kt = qk4 all j,
                # qt = qk0 icx0, all 16 V tiles) comes out of the engine
                # queues first; later passes' needs follow in pass order
                emit_transposes(range(0, 4))
                emit_k(4, [0])
                emit_q(0, [0])
                emit_transposes(range(4, 8))
                emit_k(4, [1])
                for t in range(0, 4):
                    emit_vproj(t)
                emit_transposes(range(8, 12))
                emit_k(4, [2])
                for t in range(4, 8):
                    emit_vproj(t)
                emit_transposes(range(12, NT))
                emit_k(4, [3])
                for t in range(8, NT):
                    emit_vproj(t)
                emit_k(5, [0, 1, 2, 3])
                emit_q(1, [0])
                emit_k(6, [0, 1, 2, 3])
                emit_q(2, [0])
                emit_k(7, [0, 1, 2, 3])
                emit_q(3, [0])
                for icx in (1, 2, 3):
                    for ct in range(4):
                        emit_q(ct, [icx])

                def emit_oproj(tiles):
                    for t in tiles:
                        op = ring_psum()
                        for ct in range(4):
                            nc.tensor.matmul(
                                op[0:P, 0:D], zT[ct][:, t * P:(t + 1) * P],
                                wout_sb[:, ct, :],
                                start=(ct == 0), stop=(ct == 3),
                                skip_group_check=True)
                        ot = outp.tile([P, D], F32, tag="ot", name="ot")
                        nc.vector.tensor_add(ot[:], op[0:P, 0:D], b_bc[:])
                        # permuted output rows -> 256B descriptors; spread
                        # across queues so the writes overlap compute
                        eng = (nc.sync, nc.gpsimd, nc.scalar)[t % 3]
                        eng.dma_start(
                            out_d.ap().rearrange("(p t) d -> p t d",
                                                 p=P)[:, t, :],
                            ot[:])

                # ---- passes: one (icx, head-pair) at a time ----------
                passes = [(icx, hp) for icx in range(IC)
                          for hp in range(H // 2)]
                pend = []      # [(rhs0, rhs1, j, hp, za), ...]
                slot = [0]

                def flush_av(lag=2, all_=False):
                    while pend and (all_ or len(pend) > lag):
                        rhs, j_p, hp_p, za_p = pend.pop(0)
                        for hh in (0, 1):
                            nc.tensor.matmul(
                                za_p[hh][:],
                                vt_sb[j_p][:, 2 * hp_p + hh, :],
                                rhs[hh],
                                start=(j_p == 0), stop=(j_p == NT - 1),
                                skip_group_check=True)

                def emit_norm(za_p, hp_p, icx_p):
                    # evict za to SBUF fast (one copy per engine) so the
                    # next pass's A@V can overwrite za immediately; the
                    # recip+broadcast+normalize chain runs lazily with
                    # the multiply on gpsimd (SBUF only)
                    zu = [sm_pool.tile([P, 512], F32, tag=f"zu{hh}",
                                       name=f"zu{hh}", bufs=2)
                          for hh in (0, 1)]
                    nc.vector.tensor_copy(zu[0][:], za_p[0][:])
                    nc.scalar.copy(zu[1][:], za_p[1][:])
                    for hh in (0, 1):
                        rc = sm_pool.tile([1, 512], F32, tag="rc",
                                          name="rc", bufs=2)
                        nc.vector.reciprocal_approx_fast(
                            rc[:], zu[hh][0:1, :])
                        bc = sm_pool.tile([P, 512], F32, tag="bc",
                                          name="bc", bufs=2)
                        nc.gpsimd.partition_broadcast(bc[:], rc[:])
                        nc.gpsimd.tensor_mul(
                            zT[hp_p][hh * 64:hh * 64 + 64,
                                     icx_p * 512:(icx_p + 1) * 512],
                            zu[hh][64:P, :], bc[64:P, :])

                prev = [None]   # (za, hp, icx) of the previous pass

                for px, (icx, hp) in enumerate(passes):
                    qt = qk_sb[hp]
                    kt = qk_sb[4 + hp]
                    za = [zpsum.tile([P, 512], F32, tag=f"za{hh}",
                                     name=f"za{hh}", bufs=1)
                          for hh in (0, 1)]
                    for j in range(NT):
                        flush_av(lag=2)
                        if j == 2:
                            if prev[0] is not None:
                                emit_norm(*prev[0])
                            prev[0] = (za, hp, icx)
                            # oproj for icx i once all 4 head-pairs of
                            # icx i are normalized (we are in pass
                            # (i+1, hp=1) by then)
                            if px % 4 == 1 and px >= 5:
                                emit_oproj(range(4 * (px // 4 - 1),
                                                 4 * (px // 4 - 1) + 4))
                        st = ring_psum()
                        for hh in (0, 1):
                            r0 = hh * 64
                            nc.tensor.matmul(
                                st[:, hh * 512:(hh + 1) * 512],
                                kt[r0:r0 + 64, j * P:(j + 1) * P],
                                qt[r0:r0 + 64,
                                   icx * 512:(icx + 1) * 512],
                                start=True, stop=True)
                        if slot[0] % 2 == 0:
                            es = es_pool.tile([P, 1024], BF16,
                                              tag="es", name="es",
                                              bufs=8)
                            nc.scalar.activation(
                                es[:], st[:],
                                mybir.ActivationFunctionType.Exp,
                                scale=1.0 / EXP_A)
                            rhs = (es[:, 0:512], es[:, 512:1024])
                        else:
                            # Schraudolph bf16 exp: single-op add with
                            # int16 convert-on-write (Q pre-scaled)
                            es16 = es_pool.tile([P, 1024], I16,
                                                tag="es16", name="es16",
                                                bufs=8)
                            nc.vector.tensor_scalar_add(
                                es16[:], st[:], EXP_C)
                            ebf = es16[:].bitcast(BF16)
                            rhs = (ebf[:, 0:512], ebf[:, 512:1024])
                        pend.append((rhs, j, hp, za))
                        slot[0] += 1

                flush_av(all_=True)
                emit_norm(*prev[0])
                emit_oproj(range(12, 16))

    nc.compile()
    return nc


_PROG = None


def _get_program():
    global _PROG
    if _PROG is None:
        _PROG = build_program()
    return _PROG


def kernel(x, W_qkv, W_out, b_out):
    nc = _get_program()
    x = np.asarray(x, dtype=np.float32)
    wq = np.ascontiguousarray(np.asarray(W_qkv, dtype=np.float32))
    wo = np.ascontiguousarray(np.asarray(W_out, dtype=np.float32))
    bo = np.ascontiguousarray(np.asarray(b_out, dtype=np.float32))
    in_maps = [
        {"x": np.ascontiguousarray(x[i]), "w_qkv": wq, "w_out": wo,
         "b_out": bo}
        for i in range(B)
    ]
    res = run_bass_kernel_spmd(nc, in_maps, list(range(B)))
    return np.stack([res.results[i]["out"] for i in range(B)], axis=0)


# revision 14
# speedup vs baseline: 1.0220x; 1.0220x over previous
"""Multi-head attention (B=8, N=2048, dim=64, heads=8) on 8 Trainium2 cores.

Sharding: batch-parallel — one batch element per NeuronCore, weights
replicated, no collectives. Per-core flash-style attention, fully
SBUF-resident (no HBM intermediates).

Schedule highlights (v3 — single-stream, 3-deep S-tile ring):
- One (head-pair, icx) pass at a time; S tiles rotate through a 3-deep
  PSUM ring (6 banks), so S(k) only waits on exp(k-3): ~2100ns of slack
  vs the ~1900ns S -> exp -> st-free round trip that capped the old
  2-stream layout (2-slot slack = ~1314ns -> 260ns stall per slot).
- za accumulators single-buffered (2 banks, 8 total). At each pass end
  the two za banks are EVICTED to SBUF immediately (one copy on ACT,
  one on DVE, ~540ns each) so the next pass's A@V never waits on the
  full recip+broadcast+normalize chain; that chain runs lazily with
  the normalize multiply on the otherwise-idle GPSIMD engine (SBUF in,
  SBUF out).
- Q^T tiles are pre-scaled by EXP_A*SCALE at the PSUM->SBUF copy
  (free: activation Copy with scale), so the DVE Schraudolph exp is a
  single-op tensor_scalar_add (int16 convert-on-write whose bits ARE
  the bf16 exp); ACT exp uses scale=1/EXP_A. Exp alternates engines
  per slot, ~52% on ACT.
- The softmax denominator rides the A@V matmul as a leading ones
  column of V (PSUM partition 0); z is scaled out of the SBUF-evicted
  copy by gpsimd.
- Tokens processed in a permuted order so the x load runs at 4KB per
  DMA descriptor; the output DMA applies the inverse permutation.
"""
import sys

import numpy as np


def _ensure_path():
    try:
        import concourse  # noqa: F401
    except ImportError:
        for p in (
            "/opt/trn_rl_repo",
            "/root/.axon_site",
            "/root/.axon_site/_ro/trn_rl_repo",
            "/root/.axon_site/_ro/pypackages",
        ):
            if p not in sys.path:
                sys.path.append(p)


_ensure_path()

import concourse.bacc as bacc  # noqa: E402
import concourse.mybir as mybir  # noqa: E402
import concourse.tile as tile  # noqa: E402
from concourse.bass_utils import run_bass_kernel_spmd  # noqa: E402
from concourse.masks import make_identity  # noqa: E402

B, N, D, H = 8, 2048, 64, 8
P = 128
NT = N // P          # 16 n-tiles of 128
IC = N // 512        # 4 query chunks of 512
SCALE = float(D) ** -0.5
F32 = mybir.dt.float32
F32R = mybir.dt.float32r
BF16 = mybir.dt.bfloat16
# Schraudolph bf16 exp on DVE: int16(x*EXP_A + EXP_C) bit-viewed as
# bf16 equals exp(x) to within ~3%; the softmax ratio cancels most of
# it. EXP_C = bf16 exponent bias (127<<7) minus a centering shift that
# zeroes the mean relative error so ACT-exact and DVE-approx key tiles
# mix without bias. The int16 convert-on-write does the rounding.
# Q^T is pre-scaled by EXP_A*SCALE, so S' = EXP_A*SCALE*(q.k): the DVE
# exp is int16(S' + EXP_C) in ONE ALU op; ACT exp is exp(S'/EXP_A).
EXP_A = 128.0 / float(np.log(2.0))
EXP_C = 16256.0 - 6.55
I16 = mybir.dt.int16


def build_program(n_cores=B):
    nc = bacc.Bacc("TRN2", target_bir_lowering=False, debug=False,
                   num_devices=n_cores)
    x_d = nc.dram_tensor("x", [N, D], F32, kind="ExternalInput")
    wqkv_d = nc.dram_tensor("w_qkv", [D, 3 * H * D], F32, kind="ExternalInput")
    wout_d = nc.dram_tensor("w_out", [H * D, D], F32, kind="ExternalInput")
    bout_d = nc.dram_tensor("b_out", [D], F32, kind="ExternalInput")
    out_d = nc.dram_tensor("out", [N, D], F32, kind="ExternalOutput")

    with tile.TileContext(nc) as tc:
        with tc.tile_pool(name="const", bufs=1) as const:
            # identity FIRST on gpsimd (transposes block on it); x loads
            # go on the other queues so ident isn't stuck behind them
            ident = const.tile([P, P], F32, tag="ident")
            make_identity(nc, ident[:])

            xall = const.tile([P, NT, D], F32, tag="xall")
            xr = x_d.ap().rearrange("(p t) d -> p t d", p=P)
            for qi, eng in enumerate((nc.sync, nc.gpsimd, nc.scalar,
                                      nc.sync)):
                eng.dma_start(xall[:, 4 * qi:4 * qi + 4, :],
                              xr[:, 4 * qi:4 * qi + 4, :])

            wsb = const.tile([D, 3 * H * D], F32R, tag="wqkv")
            nc.gpsimd.dma_start(wsb[:], wqkv_d.ap())
            wout_f32 = const.tile([P, 4, D], F32, tag="woutf")
            nc.gpsimd.dma_start(
                wout_f32[:], wout_d.ap().rearrange("(t p) d -> p t d", p=P))
            wout_sb = const.tile([P, 4, D], BF16, tag="wout")
            nc.vector.tensor_copy(wout_sb[:], wout_f32[:])
            b_row = const.tile([1, D], F32, tag="brow")
            nc.sync.dma_start(b_row[:], bout_d.ap().rearrange("(a d) -> a d", a=1))
            b_bc = const.tile([P, D], F32, tag="bbc")
            nc.gpsimd.partition_broadcast(b_bc[:], b_row[:])
            ones3 = const.tile([P, H, 1], F32, tag="ones3")
            nc.gpsimd.memset(ones3[:], 1.0)

            xT = const.tile([D, N], F32R, tag="xT")
            # qk_sb[0..3]: Q^T head-pairs [128, N] PRE-SCALED by
            # EXP_A*SCALE; qk_sb[4..7]: K^T pairs (unscaled)
            qk_sb = [const.tile([P, N], BF16, tag=f"qk{i}", name=f"qk{i}")
                     for i in range(8)]
            # V~ per n-tile: [128, H, 128]; per head: col 0 = ones (so the
            # softmax denominator lands at PSUM partition 0), cols 1-63
            # zero, cols 64-127 = V (z at partitions 64-127). The extra
            # LDWEIGHTS columns hide under the 512-col A@V streams via
            # the PE load-ahead window.
            vt_sb = [const.tile([P, H, P], BF16, tag=f"vt{t}", name=f"vt{t}")
                     for t in range(NT)]
            zT = [const.tile([P, N], BF16, tag=f"zT{i}", name=f"zT{i}")
                  for i in range(4)]

            with (
                tc.tile_pool(name="spsum", bufs=3,
                             space=bacc.bass.MemorySpace.PSUM) as spsum,
                tc.tile_pool(name="zpsum", bufs=1,
                             space=bacc.bass.MemorySpace.PSUM) as zpsum,
                tc.tile_pool(name="es", bufs=8) as es_pool,
                tc.tile_pool(name="sm", bufs=3) as sm_pool,
                tc.tile_pool(name="outp", bufs=3) as outp,
            ):
                # (tokens are processed in the permuted order pi(p,t) =
                # row p*NT+t: each SBUF partition loads one contiguous 4KB
                # block of x. Attention is permutation-equivariant over
                # tokens; the output DMA applies the inverse permutation.)

                def ring_psum():
                    # the 3-deep S-tile ring (6 banks); setup matmuls and
                    # the output projection borrow slots from it
                    return spsum.tile([P, 1024], F32, tag="st",
                                      name="st", bufs=3)

                def emit_k(ct, icxs):
                    w_sl = wsb[:, ct * P:(ct + 1) * P]
                    for icx in icxs:
                        mp = ring_psum()
                        nc.tensor.matmul(
                            mp[0:P, 0:512], w_sl,
                            xT[:, icx * 512:(icx + 1) * 512],
                            start=True, stop=True)
                        nc.scalar.copy(
                            qk_sb[ct][:, icx * 512:(icx + 1) * 512],
                            mp[0:P, 0:512])

                def emit_q(ct, icxs):
                    w_sl = wsb[:, ct * P:(ct + 1) * P]
                    for icx in icxs:
                        mp = ring_psum()
                        nc.tensor.matmul(
                            mp[0:P, 0:512], w_sl,
                            xT[:, icx * 512:(icx + 1) * 512],
                            start=True, stop=True)
                        # pre-scale Q by EXP_A*SCALE (free via activation
                        # Copy scale); DVE exp then needs no multiply
                        nc.scalar.activation(
                            qk_sb[ct][:, icx * 512:(icx + 1) * 512],
                            mp[0:P, 0:512],
                            mybir.ActivationFunctionType.Copy,
                            scale=EXP_A * SCALE)

                def emit_vproj(t):
                    nc.gpsimd.memset(vt_sb[t][:, :, 1:64], 0.0)
                    mp = ring_psum()
                    nc.tensor.matmul(
                        mp[0:P, 0:512], xT[:, t * P:(t + 1) * P],
                        wsb[:, 2 * H * D:3 * H * D],
                        start=True, stop=True)
                    nc.gpsimd.tensor_copy(vt_sb[t][:, :, 0:1], ones3[:])
                    nc.scalar.copy(
                        vt_sb[t][:, :, 64:P],
                        mp[0:P, 0:512].rearrange("p (h d) -> p h d", h=H))

                def emit_transposes(ts):
                    for t in ts:
                        pp = ring_psum()
                        nc.tensor.transpose(pp[0:D, 0:P], xall[:, t, :],
                                            ident[:])
                        nc.vector.tensor_copy(xT[:, t * P:(t + 1) * P],
                                              pp[0:D, 0:P])

                # setup, ordered so pass 0 ((icx0, hp0): kt = qk4 all j,
                # qt = qk0 icx0, all 16 V tiles) comes out of the engine
                # queues first; later passes' needs follow in pass order
                emit_transposes(range(0, 4))
                emit_k(4, [0])
                emit_q(0, [0])
                emit_transposes(range(4, 8))
                emit_k(4, [1])
                for t in range(0, 4):
                    emit_vproj(t)
                emit_transposes(range(8, 12))
                emit_k(4, [2])
                for t in range(4, 8):
                    emit_vproj(t)
                emit_transposes(range(12, NT))
                emit_k(4, [3])
                for t in range(8, NT):
                    emit_vproj(t)
                emit_k(5, [0, 1, 2, 3])
                emit_q(1, [0])
                emit_k(6, [0, 1, 2, 3])
                emit_q(2, [0])
                emit_k(7, [0, 1, 2, 3])
                emit_q(3, [0])
                for icx in (1, 2, 3):
                    for ct in range(4):
                        emit_q(ct, [icx])

                def emit_oproj(tiles):
                    for t in tiles:
                        op = ring_psum()
                        for ct in range(4):
                            nc.tensor.matmul(
                                op[0:P, 0:D], zT[ct][:, t * P:(t + 1) * P],
                                wout_sb[:, ct, :],
                                start=(ct == 0), stop=(ct == 3),
                                skip_group_check=True)
                        ot = outp.tile([P, D], F32, tag="ot", name="ot")
                        nc.vector.tensor_add(ot[:], op[0:P, 0:D], b_bc[:])
                        # permuted output rows -> 256B descriptors; spread
                        # across queues so the writes overlap compute
                        eng = (nc.sync, nc.gpsimd, nc.scalar)[t % 3]
                        eng.dma_start(
                            out_d.ap().rearrange("(p t) d -> p t d",
                                                 p=P)[:, t, :],
                            ot[:])

                # ---- passes: one (icx, head-pair) at a time ----------
                passes = [(icx, hp) for icx in range(IC)
                          for hp in range(H // 2)]
                pend = []      # [(rhs0, rhs1, j, hp, za), ...]
                slot = [0]

                def flush_av(lag=2, all_=False):
                    while pend and (all_ or len(pend) > lag):
                        rhs, j_p, hp_p, za_p = pend.pop(0)
                        for hh in (0, 1):
                            nc.tensor.matmul(
                                za_p[hh][:],
                                vt_sb[j_p][:, 2 * hp_p + hh, :],
                                rhs[hh],
                                start=(j_p == 0), stop=(j_p == NT - 1),
                                skip_group_check=True)

                def emit_evict(za_p):
                    # evict za to SBUF fast (ACT; DVE is the loaded
                    # engine) so the next pass's A@V can overwrite za
                    # without waiting for the normalize chain
                    zu = [sm_pool.tile([P, 512], F32, tag=f"zu{hh}",
                                       name=f"zu{hh}", bufs=2)
                          for hh in (0, 1)]
                    nc.scalar.copy(zu[0][:], za_p[0][:])
                    nc.scalar.copy(zu[1][:], za_p[1][:])
                    return zu

                def emit_norm(zu, hp_p, icx_p):
                    # lazy normalize chain: recip on DVE (small), then
                    # broadcast + multiply on the otherwise-idle gpsimd
                    # (SBUF in/out; zT needed only ~2 passes later)
                    for hh in (0, 1):
                        rc = sm_pool.tile([1, 512], F32, tag="rc",
                                          name="rc", bufs=2)
                        nc.vector.reciprocal_approx_fast(
                            rc[:], zu[hh][0:1, :])
                        bc = sm_pool.tile([P, 512], F32, tag="bc",
                                          name="bc", bufs=2)
                        nc.gpsimd.partition_broadcast(bc[:], rc[:])
                        nc.gpsimd.tensor_mul(
                            zT[hp_p][hh * 64:hh * 64 + 64,
                                     icx_p * 512:(icx_p + 1) * 512],
                            zu[hh][64:P, :], bc[64:P, :])

                prev = [None]   # (za, hp, icx) of the previous pass

                for px, (icx, hp) in enumerate(passes):
                    qt = qk_sb[hp]
                    kt = qk_sb[4 + hp]
                    za = [zpsum.tile([P, 512], F32, tag=f"za{hh}",
                                     name=f"za{hh}", bufs=1)
                          for hh in (0, 1)]
                    for j in range(NT):
                        flush_av(lag=2)
                        if j == 2 and prev[0] is not None:
                            # prev pass's A@Vs all flushed: evict its za
                            za_p, hp_p, icx_p = prev[0]
                            prev[0] = (emit_evict(za_p), hp_p, icx_p)
                        if j == 5:
                            if prev[0] is not None:
                                emit_norm(*prev[0])
                            prev[0] = (za, hp, icx)
                        if j == 8 and px % 4 == 1 and px >= 5:
                            # oproj for icx i once all 4 head-pairs of
                            # icx i are normalized (we are in pass
                            # (i+1, hp=1) by then)
                            emit_oproj(range(4 * (px // 4 - 1),
                                             4 * (px // 4 - 1) + 4))
                        st = ring_psum()
                        for hh in (0, 1):
                            r0 = hh * 64
                            nc.tensor.matmul(
                                st[:, hh * 512:(hh + 1) * 512],
                                kt[r0:r0 + 64, j * P:(j + 1) * P],
                                qt[r0:r0 + 64,
                                   icx * 512:(icx + 1) * 512],
                                start=True, stop=True)
                        if slot[0] % 2 == 0:
                            es = es_pool.tile([P, 1024], BF16,
                                              tag="es", name="es",
                                              bufs=8)
                            nc.scalar.activation(
                                es[:], st[:],
                                mybir.ActivationFunctionType.Exp,
                                scale=1.0 / EXP_A)
                            rhs = (es[:, 0:512], es[:, 512:1024])
                        else:
                            # Schraudolph bf16 exp: single-op add with
                            # int16 convert-on-write (Q pre-scaled)
                            es16 = es_pool.tile([P, 1024], I16,
                                                tag="es16", name="es16",
                                                bufs=8)
                            nc.vector.tensor_scalar_add(
                                es16[:], st[:], EXP_C)
                            ebf = es16[:].bitcast(BF16)
                            rhs = (ebf[:, 0:512], ebf[:, 512:1024])
                        pend.append((rhs, j, hp, za))
                        slot[0] += 1

                flush_av(all_=True)
                za_p, hp_p, icx_p = prev[0]
                emit_norm(emit_evict(za_p), hp_p, icx_p)
                emit_oproj(range(12, 16))

    nc.compile()
    return nc


_PROG = None


def _get_program():
    global _PROG
    if _PROG is None:
        _PROG = build_program()
    return _PROG


def kernel(x, W_qkv, W_out, b_out):
    nc = _get_program()
    x = np.asarray(x, dtype=np.float32)
    wq = np.ascontiguousarray(np.asarray(W_qkv, dtype=np.float32))
    wo = np.ascontiguousarray(np.asarray(W_out, dtype=np.float32))
    bo = np.ascontiguousarray(np.asarray(b_out, dtype=np.float32))
    in_maps = [
        {"x": np.ascontiguousarray(x[i]), "w_qkv": wq, "w_out": wo,
         "b_out": bo}
        for i in range(B)
    ]
    res = run_bass_kernel_spmd(nc, in_maps, list(range(B)))
    return np.stack([res.results[i]["out"] for i in range(B)], axis=0)
